# revision 14
# baseline (speedup 1.0000x reference)
"""MyViT (12-layer, D=768, H=12, S=197, batch 32) on 8 TRN2 NeuronCores.

Strategy: pure data parallelism — 4 images per core, weights replicated.
Inside each core the residual stream x is kept token-major in fp32; matmul
operands are fp16 (fp32 PSUM accumulation).  Per layer:

  A) LN1 (bn_stats) -> hn (fp16, token-major) -> PE-transpose -> hnT [768,T]
  B) per head: qT/kT = Wq/Wk @ hnT (feature-major), per image:
     v = hnT.T @ Wv (token-major, +ones column), scoresT = k @ qT
     (keys on partitions), expT = exp(scoresT)  [no max subtraction —
     scores are O(1) by construction], o_aug = [v|1].T @ expT giving
     oT [64,T] feature-major plus the softmax row-sums.
  C) transpose oT back to token-major, x += o * (1/rowsum) fused on DVE
  D) LN2 -> h2T (as A)
  E) MLP: y1T = W1.T @ h2T -> gelu(+b1) -> gT; y2 = gT.T @ W2, x += y2.
     Done in two M-halves to halve the gT SBUF footprint.

LN affine params are folded into the downstream weights on the host;
1/sqrt(dh) is folded into Wq; patch-embed bias / cls token / pos-emb are
folded into one additive "posfull" tensor.
"""

import numpy as np

# ---- model dims (hardcoded per the harness contract) ----
NCORES = 8
NI = 4            # images per core
S = 197           # tokens per image
T = NI * S        # 788 tokens per core
NT = 7            # token chunks of 128
TPAD = NT * 128   # 896
D = 768
ND = 6            # feature chunks of 128
H = 12
DH = 64
MLP = 3072
NM = 24           # mlp chunks of 128
CIN = 768         # 3*16*16
NKIN = 6          # CIN/128
L = 12
EPS = 1e-5

_PROG_CACHE = {}


def _tok_rows(c):
    return 128 if c < NT - 1 else T - 128 * (NT - 1)


def build_vit(n_layers=L, bv_nonzero=False, b2_nonzero=False):
    import concourse.bass as bass
    import concourse.mybir as mybir
    from concourse import bacc
    from concourse.tile import TileContext
    from concourse.masks import make_identity

    F32 = mybir.dt.float32
    F16 = mybir.dt.float16
    AF = mybir.ActivationFunctionType
    OP = mybir.AluOpType

    nc = bacc.Bacc("TRN2", target_bir_lowering=False)

    pt_d = nc.dram_tensor("pt", [NKIN, 128, T], F16, kind="ExternalInput")
    wmap_d = nc.dram_tensor("wmap", [NKIN, 128, D], F16, kind="ExternalInput")
    pos_d = nc.dram_tensor("posfull", [NT, 128, D], F32, kind="ExternalInput")
    qkvw_d = nc.dram_tensor("qkvw", [n_layers, 128, 6, 3, DH], F16, kind="ExternalInput")
    qkvb_d = nc.dram_tensor("qkvb", [n_layers, DH, H, 2], F32, kind="ExternalInput")
    w1_d = nc.dram_tensor("w1", [n_layers, ND, 128, MLP], F16, kind="ExternalInput")
    b1_d = nc.dram_tensor("b1t", [n_layers, 128, NM], F32, kind="ExternalInput")
    w2_d = nc.dram_tensor("w2", [n_layers, NM, 128, D], F16, kind="ExternalInput")
    wout_d = nc.dram_tensor("woutbc", [NI, D], F32, kind="ExternalInput")
    if bv_nonzero:
        bv_d = nc.dram_tensor("bvrow", [n_layers, 1, H, DH], F16, kind="ExternalInput")
    if b2_nonzero:
        b2_d = nc.dram_tensor("b2row", [n_layers, 1, D], F16, kind="ExternalInput")
    out_d = nc.dram_tensor("out4", [NI, 1], F32, kind="ExternalOutput")

    with TileContext(nc) as tc:
        with tc.tile_pool(name="sb", bufs=1) as sb, \
             tc.tile_pool(name="ps", space="PSUM", bufs=1) as psp:

            # ---- constants / persistent tiles
            id16 = sb.tile([128, 128], F16, tag="id16", bufs=1)
            make_identity(nc, id16)
            eps_t = sb.tile([128, 1], F32, tag="eps", bufs=1)
            nc.vector.memset(eps_t, EPS)
            if bv_nonzero or b2_nonzero:
                ones16 = sb.tile([1, 128], F16, tag="ones", bufs=1)
                nc.vector.memset(ones16, 1.0)

            x = [sb.tile([128, D], F32, tag=f"x{c}", bufs=1, name=f"x{c}")
                 for c in range(NT)]

            # ---- patch embed: x = posfull + P @ W_map
            # pt tiles borrow the gT/w1 tags, wmap tiles borrow the w2 tags.
            ptsb = [sb.tile([128, T], F16,
                            tag=(f"g{k}" if k < 12 else f"w1k{k - 12}"),
                            bufs=1, name=f"pt{k}")
                    for k in range(NKIN)]
            wmsb = [sb.tile([128, D], F16, tag=f"w2k{k}", bufs=1, name=f"wm{k}")
                    for k in range(NKIN)]
            for k in range(NKIN):
                nc.sync.dma_start(out=ptsb[k], in_=pt_d[k])
                nc.sync.dma_start(out=wmsb[k], in_=wmap_d[k])
            for c in range(NT):
                nc.sync.dma_start(out=x[c], in_=pos_d[c])
            for c in range(NT):
                mc = _tok_rows(c)
                ps = psp.tile([128, D], F32, tag="big", bufs=2, name="pe_ps")
                for s0, s1 in ((0, 512), (512, D)):
                    for k in range(NKIN):
                        nc.tensor.matmul(ps[0:mc, s0:s1],
                                         ptsb[k][:, 128 * c:128 * c + mc],
                                         wmsb[k][:, s0:s1],
                                         start=(k == 0), stop=(k == NKIN - 1))
                nc.vector.tensor_add(out=x[c][0:mc], in0=x[c][0:mc], in1=ps[0:mc])

            # LN over feature dim: 768 = 3 x 256 bn_stats subgroups
            def ln_transpose(dst, scope):
                with nc.named_scope(scope):
                    _ln_transpose(dst)

            def _ln_transpose(dst):
                for c in range(NT):
                    stats = sb.tile([128, 3, 6], F32, tag="bnst", bufs=3, name="bnst")
                    for g in range(3):
                        nc.vector.bn_stats(out=stats[:, g, :],
                                           in_=x[c][:, 256 * g:256 * (g + 1)])
                    mv = sb.tile([128, 2], F32, tag="bnmv", bufs=3, name="bnmv")
                    nc.vector.bn_aggr(out=mv, in_=stats)
                    rstd = sb.tile([128, 1], F32, tag="rstd", bufs=3, name="rstd")
                    nc.scalar.activation(out=rstd, in_=mv[:, 1:2], func=AF.Sqrt,
                                         bias=eps_t, scale=1.0)
                    nc.vector.reciprocal(out=rstd, in_=rstd)
                    hn = sb.tile([128, D], F16, tag="hn", bufs=3, name="hn")
                    nc.vector.tensor_scalar(out=hn, in0=x[c],
                                            scalar1=mv[:, 0:1], scalar2=rstd,
                                            op0=OP.subtract, op1=OP.mult)
                    for r in range(ND):
                        tp = psp.tile([128, 128], F16, tag="small", bufs=4, name="tp")
                        nc.tensor.transpose(tp, hn[:, 128 * r:128 * (r + 1)], id16)
                        nc.any.tensor_copy(out=dst[r][:, 128 * c:128 * (c + 1)], in_=tp)

            for l in range(n_layers):
                # ---- layer weight loads
                w1sb = [sb.tile([128, MLP], F16, tag=f"w1k{k}", bufs=1, name=f"w1_{l}_{k}")
                        for k in range(ND)]
                for k in range(ND):
                    nc.sync.dma_start(out=w1sb[k], in_=w1_d[l, k])
                w2sb = [sb.tile([128, D], F16, tag=f"w2k{k}", bufs=1, name=f"w2_{l}_{k}")
                        for k in range(NM)]
                for k in range(NM):
                    nc.sync.dma_start(out=w2sb[k], in_=w2_d[l, k])
                qw = sb.tile([128, 6, 3, DH], F16, tag="qkvw", bufs=2, name=f"qw{l}")
                nc.sync.dma_start(out=qw, in_=qkvw_d[l])
                qb = sb.tile([DH, H, 2], F32, tag="qkvb", bufs=2, name=f"qb{l}")
                nc.sync.dma_start(out=qb, in_=qkvb_d[l])
                b1sb = sb.tile([128, NM], F32, tag="b1t", bufs=2, name=f"b1_{l}")
                nc.sync.dma_start(out=b1sb, in_=b1_d[l])
                if bv_nonzero:
                    bvsb = sb.tile([1, H, DH], F16, tag="bvr", bufs=2, name=f"bv{l}")
                    nc.sync.dma_start(out=bvsb, in_=bv_d[l])
                if b2_nonzero:
                    b2sb = sb.tile([1, D], F16, tag="b2r", bufs=2, name=f"b2_{l}")
                    nc.sync.dma_start(out=b2sb, in_=b2_d[l])

                # ---- A: LN1 -> hnT
                hnT = [sb.tile([128, TPAD], F16, tag=f"xt{r}", bufs=1, name=f"hnT{l}_{r}")
                       for r in range(ND)]
                ln_transpose(hnT, f"L{l}A")

                # ---- B: attention (o produced token-major, image-local
                #         chunks; normalized by the matmul-computed rowsum)
                opad = [sb.tile([128, D], F16, tag=f"op{j}", bufs=1, name=f"opad{l}_{j}")
                        for j in range(2 * NI)]
                oglob = [sb.tile([128, D], F16, tag=f"og{c}", bufs=1, name=f"og{l}_{c}")
                         for c in range(NT)]
                scope_b = nc.named_scope(f"L{l}B"); scope_b.__enter__()
                nc.any.memset(oglob[NT - 1], 0.0)

                for h in range(H):
                    b = (h % 2) * 64
                    hp = h // 2
                    r = (DH * h) // 128
                    qs = sb.tile([DH, T], F16, tag="q", bufs=2, name=f"q{l}_{h}")
                    ks = sb.tile([DH, T], F16, tag="k", bufs=2, name=f"k{l}_{h}")
                    for dst, j in ((qs, 0), (ks, 1)):
                        ps = psp.tile([DH, T], F32, tag="big", bufs=2, name="qk_ps")
                        for s0, s1 in ((0, 512), (512, T)):
                            nc.tensor.matmul(ps[:, s0:s1], qw[b:b + 64, hp, j, :],
                                             hnT[r][b:b + 64, s0:s1],
                                             start=True, stop=True)
                        nc.scalar.activation(out=dst, in_=ps, func=AF.Identity,
                                             bias=qb[:, h, j:j + 1], scale=1.0)
                    for i in range(NI):
                        t0 = S * i
                        vt = sb.tile([128, 2, 65], F16, tag="v", bufs=4, name="vt")
                        nc.any.memset(vt, 1.0)
                        vps = psp.tile([128, 2, DH], F32, tag="small", bufs=4, name="v_ps")
                        for kc, ksz in ((0, 128), (1, 69)):
                            nc.tensor.matmul(
                                vps[0:ksz, kc, :],
                                hnT[r][b:b + 64, t0 + 128 * kc: t0 + 128 * kc + ksz],
                                qw[b:b + 64, hp, 2, :],
                                start=True, stop=not bv_nonzero)
                            if bv_nonzero:
                                nc.tensor.matmul(vps[0:ksz, kc, :],
                                                 ones16[0:1, 0:ksz], bvsb[0:1, h, :],
                                                 start=False, stop=True)
                            nc.any.tensor_copy(out=vt[0:ksz, kc, 0:DH],
                                               in_=vps[0:ksz, kc, :])
                        scps = psp.tile([128, 2, S], F32, tag="small", bufs=4, name="sc_ps")
                        for kc, ksz in ((0, 128), (1, 69)):
                            nc.tensor.matmul(scps[0:ksz, kc, :],
                                             ks[:, t0 + 128 * kc: t0 + 128 * kc + ksz],
                                             qs[:, t0:t0 + S],
                                             start=True, stop=True)
                        ex = sb.tile([128, 2, S], F16, tag="ex", bufs=4, name="ex")
                        for kc, ksz in ((0, 128), (1, 69)):
                            nc.scalar.activation(out=ex[0:ksz, kc, :],
                                                 in_=scps[0:ksz, kc, :], func=AF.Exp)
                        ops_ = psp.tile([128, 2, 65], F32, tag="small", bufs=4, name="o_ps")
                        for qc, qsz in ((0, 128), (1, 69)):
                            for kc, ksz in ((0, 128), (1, 69)):
                                nc.tensor.matmul(
                                    ops_[0:qsz, qc, :],
                                    ex[0:ksz, kc, 128 * qc:128 * qc + qsz],
                                    vt[0:ksz, kc, :],
                                    start=(kc == 0), stop=(kc == 1))
                        rec = sb.tile([128, 2, 1], F32, tag="rec", bufs=4, name="rec")
                        for qc, qsz in ((0, 128), (1, 69)):
                            nc.vector.reciprocal(out=rec[0:qsz, qc, :],
                                                 in_=ops_[0:qsz, qc, 64:65])
                            nc.vector.tensor_scalar(
                                out=opad[2 * i + qc][0:qsz, DH * h:DH * h + DH],
                                in0=ops_[0:qsz, qc, 0:DH],
                                scalar1=rec[0:qsz, qc, :], scalar2=None,
                                op0=OP.mult)

                # repack image-local (197-token) chunks to global 128-chunks
                for i in range(NI):
                    for qc, qsz in ((0, 128), (1, 69)):
                        g0 = S * i + 128 * qc          # global start token
                        left = qsz
                        src0 = 0
                        while left > 0:
                            c, p = (g0 + src0) // 128, (g0 + src0) % 128
                            n = min(left, 128 - p)
                            nc.sync.dma_start(
                                out=oglob[c][p:p + n, :],
                                in_=opad[2 * i + qc][src0:src0 + n, :])
                            src0 += n
                            left -= n
                scope_b.__exit__(None, None, None)
                # ---- C: x += o
                scope_c = nc.named_scope(f"L{l}C"); scope_c.__enter__()
                for c in range(NT):
                    nc.vector.tensor_add(out=x[c], in0=x[c], in1=oglob[c])
                scope_c.__exit__(None, None, None)
                # ---- D: LN2 -> h2T
                h2T = [sb.tile([128, TPAD], F16, tag=f"xt{r}", bufs=1, name=f"h2T{l}_{r}")
                       for r in range(ND)]
                ln_transpose(h2T, f"L{l}D")

                # ---- E: MLP in two M-halves (gT footprint / 2)
                scope_e = nc.named_scope(f"L{l}E"); scope_e.__enter__()
                for half in range(2):
                    m0 = NM // 2 * half
                    gT = [sb.tile([128, T], F16, tag=f"g{m}", bufs=1, name=f"gT{l}_{m0 + m}")
                          for m in range(NM // 2)]
                    for m in range(NM // 2):
                        ma = m0 + m
                        ps = psp.tile([128, T], F32, tag="big", bufs=2, name="y1_ps")
                        for s0, s1 in ((0, 512), (512, T)):
                            for k in range(ND):
                                nc.tensor.matmul(ps[:, s0:s1],
                                                 w1sb[k][:, 128 * ma:128 * (ma + 1)],
                                                 h2T[k][:, s0:s1],
                                                 start=(k == 0), stop=(k == ND - 1))
                        nc.scalar.activation(out=gT[m], in_=ps, func=AF.Gelu,
                                             bias=b1sb[:, ma:ma + 1], scale=1.0)
                    for c in range(NT):
                        mc = _tok_rows(c)
                        ps = psp.tile([128, D], F32, tag="big", bufs=2, name="y2_ps")
                        for s0, s1 in ((0, 512), (512, D)):
                            nk = NM // 2
                            for k in range(nk):
                                last = (k == nk - 1) and not (b2_nonzero and half == 1)
                                nc.tensor.matmul(ps[0:mc, s0:s1],
                                                 gT[k][:, 128 * c:128 * c + mc],
                                                 w2sb[m0 + k][:, s0:s1],
                                                 start=(k == 0), stop=last)
                            if b2_nonzero and half == 1:
                                nc.tensor.matmul(ps[0:mc, s0:s1],
                                                 ones16[0:1, 0:mc], b2sb[0:1, s0:s1],
                                                 start=False, stop=True)
                        nc.vector.tensor_add(out=x[c][0:mc], in0=x[c][0:mc],
                                             in1=ps[0:mc])

                scope_e.__exit__(None, None, None)
            # ---- head: out[i] = x[cls_i] . W_out
            cls = sb.tile([NI, D], F32, tag="cls", bufs=1)
            for i in range(NI):
                g = S * i
                c, p = g // 128, g % 128
                nc.sync.dma_start(out=cls[i:i + 1, :], in_=x[c][p:p + 1, :])
            wout_sb = sb.tile([NI, D], F32, tag="wout", bufs=1)
            nc.sync.dma_start(out=wout_sb, in_=wout_d[:, :])
            clst = sb.tile([NI, D], F32, tag="clst", bufs=1)
            o4 = sb.tile([NI, 1], F32, tag="o4", bufs=1)
            nc.vector.scalar_tensor_tensor(out=clst, in0=cls, scalar=1.0,
                                           in1=wout_sb, op0=OP.mult, op1=OP.mult,
                                           accum_out=o4)
            nc.sync.dma_start(out=out_d[:, :], in_=o4)

    nc.compile()
    return nc


def host_prep(inputs, n_layers=L):
    """Returns (in_maps, b_out, flags) for the 8 cores."""
    f32 = np.float32
    f16 = np.float16
    images = np.asarray(inputs["images"], f32)
    W_map = np.asarray(inputs["W_map"], f32)
    b_map = np.asarray(inputs["b_map"], f32)
    cls_tok = np.asarray(inputs["cls_tok"], f32)
    pos_emb = np.asarray(inputs["pos_emb"], f32)
    ln1_g = np.asarray(inputs["ln1_g"], f32)
    ln1_b = np.asarray(inputs["ln1_b"], f32)
    Wq = np.asarray(inputs["Wq"], f32)
    bq = np.asarray(inputs["bq"], f32)
    Wk = np.asarray(inputs["Wk"], f32)
    bk = np.asarray(inputs["bk"], f32)
    Wv = np.asarray(inputs["Wv"], f32)
    bv = np.asarray(inputs["bv"], f32)
    ln2_g = np.asarray(inputs["ln2_g"], f32)
    ln2_b = np.asarray(inputs["ln2_b"], f32)
    W1 = np.asarray(inputs["W1"], f32)
    b1 = np.asarray(inputs["b1"], f32)
    W2 = np.asarray(inputs["W2"], f32)
    b2 = np.asarray(inputs["b2"], f32)
    W_out = np.asarray(inputs["W_out"], f32)
    b_out = np.asarray(inputs["b_out"], f32)

    nimg = images.shape[0]
    assert nimg == NCORES * NI

    # patchify: (c, row, col) flatten order within each 16x16 patch
    pat = images.reshape(nimg, 3, 14, 16, 14, 16).transpose(0, 2, 4, 1, 3, 5)
    pat = pat.reshape(nimg, 196, CIN)

    pts = []
    for core in range(NCORES):
        P = np.zeros((T, CIN), f32)
        for i in range(NI):
            P[S * i + 1:S * (i + 1)] = pat[NI * core + i]
        pts.append(np.ascontiguousarray(
            P.T.reshape(NKIN, 128, T)).astype(f16))

    wmap16 = np.ascontiguousarray(W_map.reshape(NKIN, 128, D)).astype(f16)

    posf = np.zeros((TPAD, D), f32)
    blk = np.empty((S, D), f32)
    blk[0] = pos_emb[0] + cls_tok[0]
    blk[1:] = pos_emb[1:] + b_map[None, :]
    for i in range(NI):
        posf[S * i:S * (i + 1)] = blk
    posfull = np.ascontiguousarray(posf.reshape(NT, 128, D))

    scale = f32(1.0 / np.sqrt(DH))
    qkvw = np.zeros((n_layers, 128, 6, 3, DH), f32)
    qkvb = np.zeros((n_layers, DH, H, 2), f32)
    bv_eff = np.zeros((n_layers, 1, H, DH), f32)
    for l in range(n_layers):
        for h in range(H):
            sl = slice(DH * h, DH * h + DH)
            b = (h % 2) * 64
            hp = h // 2
            g1, bb = ln1_g[l][sl], ln1_b[l][sl]
            for j, W, B in ((0, Wq, bq), (1, Wk, bk)):
                We = g1[:, None] * W[l, h]
                Be = B[l, h] + bb @ W[l, h]
                if j == 0:
                    We, Be = We * scale, Be * scale
                qkvw[l, b:b + 64, hp, j] = We
                qkvb[l, :, h, j] = Be
            qkvw[l, b:b + 64, hp, 2] = g1[:, None] * Wv[l, h]
            bv_eff[l, 0, h] = bv[l, h] + bb @ Wv[l, h]

    W1e = ln2_g[:n_layers, :, None] * W1[:n_layers]
    b1e = b1[:n_layers] + np.einsum('ld,ldm->lm', ln2_b[:n_layers], W1[:n_layers])
    w1_16 = np.ascontiguousarray(W1e.reshape(n_layers, ND, 128, MLP)).astype(f16)
    b1t = np.ascontiguousarray(
        b1e.reshape(n_layers, NM, 128).transpose(0, 2, 1)).astype(f32)
    w2_16 = np.ascontiguousarray(W2[:n_layers].reshape(n_layers, NM, 128, D)).astype(f16)
    woutbc = np.ascontiguousarray(np.broadcast_to(W_out[:, 0], (NI, D))).astype(f32)

    bv_nonzero = bool(np.any(bv_eff != 0.0))
    b2_nonzero = bool(np.any(b2[:n_layers] != 0.0))

    shared = {
        "wmap": wmap16,
        "posfull": posfull,
        "qkvw": qkvw.astype(f16),
        "qkvb": qkvb,
        "w1": w1_16,
        "b1t": b1t,
        "w2": w2_16,
        "woutbc": woutbc,
    }
    if bv_nonzero:
        shared["bvrow"] = bv_eff.astype(f16)
    if b2_nonzero:
        shared["b2row"] = np.ascontiguousarray(
            b2[:n_layers].reshape(n_layers, 1, D)).astype(f16)

    in_maps = [dict(shared, pt=pts[core]) for core in range(NCORES)]
    return in_maps, b_out, (bv_nonzero, b2_nonzero)


def _run(inputs, trace=False, **run_kwargs):
    in_maps, b_out, (bvnz, b2nz) = host_prep(inputs)
    key = (L, bvnz, b2nz)
    if key not in _PROG_CACHE:
        _PROG_CACHE[key] = build_vit(L, bvnz, b2nz)
    nc = _PROG_CACHE[key]

    from concourse.bass_utils import run_bass_kernel_spmd
    res = run_bass_kernel_spmd(nc, in_maps, core_ids=list(range(NCORES)),
                               trace=trace, **run_kwargs)
    out = np.concatenate([r["out4"] for r in res.results], axis=0)
    return (out + b_out[None, :]).astype(np.float32), res


def kernel(**inputs):
    return _run(inputs)[0]
